# revision 45
# baseline (speedup 1.0000x reference)
"""Bass/Trainium2 kernel for nn_BipartiteNeuralMessagePassingLayer.

Contract: kernel(**inputs) takes the FULL (unsharded) fp32 inputs and
returns the FULL outputs (edge_latent [512,512,128], nodes_a [512,128],
nodes_b [512,128]) as np.float32, matching reference.reference().

Sharding strategy (8 NeuronCores): shard n_a (rows of the dense edge
grid) across the 8 cores — 64 rows each. edge_embeds is sharded,
nodes_b and the edge-MLP weights are replicated. Each core computes its
edge_latent shard — >99% of the model's FLOPs. The host gathers the
shards (the all-reduce of the sharding hint becomes a host-side gather
+ sum), reduces sum_a/sum_b from the assembled edge_latent, and applies
the two tiny node MLPs in exact fp32.

Device-side layout: everything runs in "transposed" feature-major space
(features on SBUF partitions) so the PE contracts feature dims without
any on-device transposes. The host pre-transposes + pre-casts the big
edge tensor to bf16 during sharding; edge_latent comes back transposed
([a, d, b], bf16) and the host restores [a, b, d] fp32.

Per a-row block (512 b's, all engines in parallel via a 1-pair-deep
software pipeline):
  h.T   = relu(Wec0.T@Xt_c0 + Wec1.T@Xt_c1 + Wb.T@nodes_b.T + aterm[a])
  out.T = relu(We2.T@h.T + be2)
with aterm = (nodes_a@Wa + be1).T precomputed on host (per-partition
bias columns). relu work alternates between ScalarE and VectorE.
"""

import os
from contextlib import ExitStack

import ml_dtypes
import numpy as np

import concourse.bass as bass
import concourse.mybir as mybir
import concourse.tile as tile
from concourse import bacc

# Problem constants (hardcoded per the task spec).
N_A = 512
N_B = 512
D = 128          # NODE_DIM == EDGE_DIM
FEAT = 256       # 2 * EDGE_DIM (edge feature dim)
P = 128          # SBUF partitions
NCORES = 8
SH = N_A // NCORES      # 64 a-rows per core
ABATCH = 4              # a-blocks per input DMA batch (1 MiB DMAs)
NBATCH = SH // ABATCH   # 16 input batches per core
GROUP = 4               # a-blocks per output staging tile / store DMA
NG = SH // GROUP        # 16 output groups
SKEW = 2                # software-pipeline depth in block pairs

BF16 = mybir.dt.bfloat16
F32 = mybir.dt.float32
AF = mybir.ActivationFunctionType

_PROGRAM = None
LAST_RESULTS = None  # BassKernelResults of the most recent run (for test.py)

# wpack layout: 4 [P,P] edge weights, then nodes_b^T [P,N_B] (all bf16)
W_NAMES = ["wec0", "wec1", "wb", "we2"]
WPACK_COLS = 4 * P + N_B


def build_program(ng: int = NG) -> bass.Bass:
    """Trace the per-core SPMD program. Identical on all 8 cores; all
    per-core differences come in via the input tensors.

    ng < NG builds a truncated variant (first ng output groups only) for
    hardware bisection; out_el keeps its full shape."""
    nc = bacc.Bacc(
        "TRN2",
        target_bir_lowering=False,
        debug=False,
        num_devices=NCORES,
    )

    # --- I/O declarations ------------------------------------------------
    # xt: edge_embeds shard, pre-transposed+bf16 on host, partition-major
    # so each DMA reads one fully-contiguous 8 KiB run per partition.
    #   [batch, p(feat%128), feat_chunk, a_in_batch, b]
    xt_d = nc.declare_dram_parameter("xt", [NBATCH, P, 2, ABATCH, N_B], BF16, isOutput=False)
    wpack_d = nc.declare_dram_parameter("wpack", [P, WPACK_COLS], BF16, isOutput=False)
    # fpack: [aterm (SH cols) | be2] fp32 per-partition bias columns.
    fpack_d = nc.declare_dram_parameter("fpack", [P, SH + 1], F32, isOutput=False)

    # Output: edge_latent shard (transposed, grouped), bf16.
    out_el_d = nc.declare_dram_parameter("out_el", [NG, P, GROUP, N_B], BF16, isOutput=True)

    with ExitStack() as ctx:
        tc = ctx.enter_context(tile.TileContext(nc))
        const = ctx.enter_context(tc.tile_pool(name="const", bufs=1))
        xp = ctx.enter_context(tc.tile_pool(name="xp", bufs=NBATCH))
        hp = ctx.enter_context(tc.tile_pool(name="hp", bufs=8))
        op = ctx.enter_context(tc.tile_pool(name="op", bufs=6))
        ps = ctx.enter_context(tc.tile_pool(name="ps", bufs=8, space="PSUM"))

        # Constants on the SP HWDGE ring (tiny, complete immediately); the
        # big edge-tensor stream on the SWDGE (gpsimd) ring so the store
        # DMAs — also on SP — don't queue behind 16 MB of loads.
        wp = const.tile([P, WPACK_COLS], BF16, tag="wp")
        nc.sync.dma_start(wp[:], wpack_d[:])
        fp = const.tile([P, SH + 1], F32, tag="fp")
        nc.sync.dma_start(fp[:], fpack_d[:])

        w_t = {n: wp[:, i * P:(i + 1) * P] for i, n in enumerate(W_NAMES)}
        nbt_t = wp[:, 4 * P:4 * P + N_B]
        aterm = fp[:, 0:SH]
        be2_t = fp[:, SH:SH + 1]

        # Loads split across two independent DMA paths (each path executes
        # its DMAs serially FIFO): first half on the ACT HWDGE ring —
        # issued at trace start, before any relu occupies ACT's queue —
        # second half on the SWDGE (gpsimd) ring. Stores get the SP ring
        # to themselves.
        xtiles = []
        for bt in range(NBATCH):
            xt_t = xp.tile([P, 2 * ABATCH * N_B], BF16, tag="xt")
            dst = xt_t[:].rearrange("p (c a b) -> p c a b", c=2, a=ABATCH, b=N_B)
            src = xt_d[bt]  # [p, c, a, b]
            eng = nc.gpsimd
            if bt == 0:
                # split the first batch so compute starts ~2 us earlier
                eng.dma_start(dst[:, :, 0:2, :], src[:, :, 0:2, :])
                eng.dma_start(dst[:, :, 2:, :], src[:, :, 2:, :])
            else:
                eng.dma_start(dst, src)
            xtiles.append(xt_t)

        # --- main loop: block pairs, software-pipelined one pair deep -----
        # Per iteration (steady state), in trace order:
        #   1. PE: mm2 for the previous pair (h ready from last iteration)
        #   2. PE: mm1 accumulation for the current pair (weights loaded
        #      once per pair: wec0 x2 blocks, wec1 x2, wb x2)
        #   3. ACT/DVE: relu2(+be2) for prev pair -> ostage (+ store DMA)
        #   4. ACT/DVE: relu1(+aterm) for the current pair
        # so TensorE streams matmuls back-to-back (stays at K=8/8) while
        # the elementwise engines chew on the previous pair. relu work
        # alternates between ScalarE and VectorE per block.
        ostages = {}

        def act_relu(dst, src, bias_col):
            nc.scalar.activation(dst, src, AF.Relu, bias=bias_col)

        def dve_relu(dst, src, bias_col):
            nc.vector.tensor_scalar(dst, src, bias_col, 0.0,
                                    mybir.AluOpType.add, mybir.AluOpType.max)

        def emit_mm2(a, h):
            ps2 = ps.tile([P, N_B], F32, tag="ps", name=f"ps2_{a}")
            nc.tensor.matmul(ps2[:], w_t["we2"], h[:], start=True, stop=True)
            return ps2

        def emit_relu2(a, ps2):
            g, j = divmod(a, GROUP)
            if g not in ostages:
                ostages[g] = op.tile([P, GROUP * N_B], BF16, tag="ost", name=f"ost{g}")
            osl = ostages[g][:, j * N_B:(j + 1) * N_B]
            (act_relu if a % 2 == 0 else dve_relu)(osl, ps2[:], be2_t)
            if j == GROUP - 1:
                # last two groups ride the idle SP ring so the tail store
                # isn't queued behind earlier ACT-ring store transfers
                eng = nc.sync if g >= ng - 2 else nc.scalar
                eng.dma_start(out_el_d[g].rearrange("p j b -> p (j b)"),
                              ostages[g][:])

        def emit_relu1(a, ps1):
            h = hp.tile([P, N_B], BF16, tag="h", name=f"h{a}")
            (act_relu if a % 2 == 1 else dve_relu)(h[:], ps1[:], aterm[:, a:a + 1])
            return h

        from collections import deque
        pending = deque()  # pairs awaiting mm2+relu2; each = [(a, h), (a, h)]

        def retire_pair():
            old = pending.popleft()
            ps2s = [emit_mm2(a, h) for a, h in old]
            return old, ps2s

        for pr in range(ng * GROUP // 2):
            a0 = 2 * pr
            # 1. a SKEW-old pair's mm2 first — PE work whose h has long
            #    been ready, so PE never waits on the elementwise engines.
            old = retire_pair() if len(pending) == SKEW else None
            # 2. current pair's mm1 accumulation, weights shared per pair.
            ps1s = [ps.tile([P, N_B], F32, tag="ps", name=f"ps1_{a0 + i}")
                    for i in range(2)]
            for wi, wname in enumerate(["wec0", "wec1"]):
                for bi, a in enumerate((a0, a0 + 1)):
                    bt, aa = divmod(a, ABATCH)
                    rhs = xtiles[bt][:, (wi * ABATCH + aa) * N_B:
                                     (wi * ABATCH + aa + 1) * N_B]
                    nc.tensor.matmul(ps1s[bi][:], w_t[wname], rhs,
                                     start=(wi == 0), stop=False)
            for bi, a in enumerate((a0, a0 + 1)):
                nc.tensor.matmul(ps1s[bi][:], w_t["wb"], nbt_t,
                                 start=False, stop=True)
            # 3. the retired pair's relu2 (+ store when a group completes).
            if old:
                for (a, _), ps2 in zip(old[0], old[1]):
                    emit_relu2(a, ps2)
            # 4. current pair's relu1.
            pending.append([(a, emit_relu1(a, ps1s[bi]))
                            for bi, a in enumerate((a0, a0 + 1))])
        # epilogue: drain the pipeline
        while pending:
            old = retire_pair()
            for (a, _), ps2 in zip(old[0], old[1]):
                emit_relu2(a, ps2)

    nc.compile()
    return nc


def _get_program() -> bass.Bass:
    global _PROGRAM
    if _PROGRAM is None:
        _PROGRAM = build_program()
    return _PROGRAM


def make_in_maps(inputs: dict) -> list[dict]:
    bf = ml_dtypes.bfloat16
    E = np.asarray(inputs["edge_embeds"], np.float32)
    na_e = np.asarray(inputs["nodes_a_embeds"], np.float32)
    nb_e = np.asarray(inputs["nodes_b_embeds"], np.float32)
    We1 = np.asarray(inputs["We1"], np.float32)
    We2 = np.asarray(inputs["We2"], np.float32)
    be1 = np.asarray(inputs["be1"], np.float32)
    be2 = np.asarray(inputs["be2"], np.float32)

    # [core, bt, aa, b, c, p] -> [core, bt, p, c, aa, b]  (partition-major)
    Ebf = np.ascontiguousarray(
        E.astype(bf).reshape(NCORES, NBATCH, ABATCH, N_B, 2, P).transpose(0, 1, 5, 4, 2, 3)
    )

    # Weights are stored [in, out]; matmul lhsT wants [K=in on partitions,
    # M=out] — exactly W as stored, so wpack columns are the raw slices.
    wpack = np.concatenate(
        [We1[256:384], We1[384:512], We1[128:256], We2, nb_e.T], axis=1
    ).astype(bf)

    # aterm[:, a] = (nodes_a @ Wa + be1).T column a — fp32 per-block bias.
    aterm_all = (na_e @ We1[0:128] + be1).astype(np.float32)  # [N_A, D]

    def fpack_for(core):
        at = aterm_all[core * SH:(core + 1) * SH].T  # [P, SH]
        return np.ascontiguousarray(
            np.concatenate([at, be2.reshape(P, 1)], axis=1), np.float32
        )

    return [{"xt": Ebf[k], "wpack": wpack, "fpack": fpack_for(k)}
            for k in range(NCORES)]


def assemble_outputs(inputs: dict, results: list[dict]):
    na_e = np.asarray(inputs["nodes_a_embeds"], np.float32)
    nb_e = np.asarray(inputs["nodes_b_embeds"], np.float32)
    Wn1 = np.asarray(inputs["Wn1"], np.float32)
    Wn2 = np.asarray(inputs["Wn2"], np.float32)
    bn1 = np.asarray(inputs["bn1"], np.float32)
    bn2 = np.asarray(inputs["bn2"], np.float32)

    edge_latent = np.empty((N_A, N_B, D), np.float32)
    for k in range(NCORES):
        el = np.asarray(results[k]["out_el"], np.float32)  # [NG, P, GROUP, N_B]
        edge_latent[k * SH:(k + 1) * SH] = (
            el.transpose(0, 2, 3, 1).reshape(SH, N_B, D)
        )

    # Node updates in exact fp32 on the assembled tensor (the cross-shard
    # sum over a IS the all-reduce from the sharding hint).
    sum_a = edge_latent.sum(axis=1)  # [N_A, D]
    sum_b = edge_latent.sum(axis=0)  # [N_B, D]

    def node_mlp(x):
        h = np.maximum(x @ Wn1 + bn1, 0.0)
        return np.maximum(h @ Wn2 + bn2, 0.0).astype(np.float32)

    nodes_a = node_mlp(np.concatenate([na_e, sum_a], axis=1))
    nodes_b = node_mlp(np.concatenate([nb_e, sum_b], axis=1))
    return edge_latent, nodes_a, nodes_b


def kernel(**inputs):
    global LAST_RESULTS
    from concourse.bass_utils import run_bass_kernel_spmd

    nc = _get_program()
    in_maps = make_in_maps(inputs)
    res = run_bass_kernel_spmd(nc, in_maps, core_ids=list(range(NCORES)))
    LAST_RESULTS = res
    return assemble_outputs(inputs, res.results)


# revision 46
# speedup vs baseline: 1.1362x; 1.1362x over previous
"""Bass/Trainium2 kernel for nn_BipartiteNeuralMessagePassingLayer.

Contract: kernel(**inputs) takes the FULL (unsharded) fp32 inputs and
returns the FULL outputs (edge_latent [512,512,128], nodes_a [512,128],
nodes_b [512,128]) as np.float32, matching reference.reference().

Sharding strategy (8 NeuronCores): shard n_a (rows of the dense edge
grid) across the 8 cores — 64 rows each. edge_embeds is sharded,
nodes_b and the edge-MLP weights are replicated. Each core computes its
edge_latent shard — >99% of the model's FLOPs. The host gathers the
shards (the all-reduce of the sharding hint becomes a host-side gather
+ sum), reduces sum_a/sum_b from the assembled edge_latent, and applies
the two tiny node MLPs in exact fp32.

Device-side layout: everything runs in "transposed" feature-major space
(features on SBUF partitions) so the PE contracts feature dims without
any on-device transposes. The host pre-transposes + pre-casts the big
edge tensor to bf16 during sharding; edge_latent comes back transposed
([a, d, b], bf16) and the host restores [a, b, d] fp32.

Per a-row block (512 b's, all engines in parallel via a 1-pair-deep
software pipeline):
  h.T   = relu(Wec0.T@Xt_c0 + Wec1.T@Xt_c1 + Wb.T@nodes_b.T + aterm[a])
  out.T = relu(We2.T@h.T + be2)
with aterm = (nodes_a@Wa + be1).T precomputed on host (per-partition
bias columns). relu work alternates between ScalarE and VectorE.
"""

import os
from contextlib import ExitStack

import ml_dtypes
import numpy as np

import concourse.bass as bass
import concourse.mybir as mybir
import concourse.tile as tile
from concourse import bacc

# Problem constants (hardcoded per the task spec).
N_A = 512
N_B = 512
D = 128          # NODE_DIM == EDGE_DIM
FEAT = 256       # 2 * EDGE_DIM (edge feature dim)
P = 128          # SBUF partitions
NCORES = 8
SH = N_A // NCORES      # 64 a-rows per core
ABATCH = 4              # a-blocks per input DMA batch (1 MiB DMAs)
NBATCH = SH // ABATCH   # 16 input batches per core
GROUP = 4               # a-blocks per output staging tile / store DMA
NG = SH // GROUP        # 16 output groups
SKEW = 2                # software-pipeline depth in block pairs

BF16 = mybir.dt.bfloat16
F32 = mybir.dt.float32
AF = mybir.ActivationFunctionType

_PROGRAM = None
LAST_RESULTS = None  # BassKernelResults of the most recent run (for test.py)

# wpack layout: 4 [P,P] edge weights, then nodes_b^T [P,N_B] (all bf16)
W_NAMES = ["wec0", "wec1", "wb", "we2"]
WPACK_COLS = 4 * P + N_B


def build_program(ng: int = NG) -> bass.Bass:
    """Trace the per-core SPMD program. Identical on all 8 cores; all
    per-core differences come in via the input tensors.

    ng < NG builds a truncated variant (first ng output groups only) for
    hardware bisection; out_el keeps its full shape."""
    nc = bacc.Bacc(
        "TRN2",
        target_bir_lowering=False,
        debug=False,
        num_devices=NCORES,
    )

    # --- I/O declarations ------------------------------------------------
    # xt: edge_embeds shard, pre-transposed+bf16 on host, partition-major
    # so each DMA reads one fully-contiguous 8 KiB run per partition.
    #   [batch, p(feat%128), feat_chunk, a_in_batch, b]
    xt_d = nc.declare_dram_parameter("xt", [NBATCH, P, 2, ABATCH, N_B], BF16, isOutput=False)
    wpack_d = nc.declare_dram_parameter("wpack", [P, WPACK_COLS], BF16, isOutput=False)
    # fpack: [aterm (SH cols) | be2] fp32 per-partition bias columns.
    fpack_d = nc.declare_dram_parameter("fpack", [P, SH + 1], F32, isOutput=False)

    # Output: edge_latent shard (transposed, grouped), bf16.
    out_el_d = nc.declare_dram_parameter("out_el", [NG, P, GROUP, N_B], BF16, isOutput=True)

    with ExitStack() as ctx:
        tc = ctx.enter_context(tile.TileContext(nc))
        const = ctx.enter_context(tc.tile_pool(name="const", bufs=1))
        xp = ctx.enter_context(tc.tile_pool(name="xp", bufs=NBATCH))
        hp = ctx.enter_context(tc.tile_pool(name="hp", bufs=8))
        op = ctx.enter_context(tc.tile_pool(name="op", bufs=6))
        ps = ctx.enter_context(tc.tile_pool(name="ps", bufs=8, space="PSUM"))

        # Constants on the SP HWDGE ring (tiny, complete immediately); the
        # big edge-tensor stream on the SWDGE (gpsimd) ring so the store
        # DMAs — also on SP — don't queue behind 16 MB of loads.
        wp = const.tile([P, WPACK_COLS], BF16, tag="wp")
        nc.sync.dma_start(wp[:], wpack_d[:])
        fp = const.tile([P, SH + 1], F32, tag="fp")
        nc.sync.dma_start(fp[:], fpack_d[:])

        w_t = {n: wp[:, i * P:(i + 1) * P] for i, n in enumerate(W_NAMES)}
        nbt_t = wp[:, 4 * P:4 * P + N_B]
        aterm = fp[:, 0:SH]
        be2_t = fp[:, SH:SH + 1]

        # Loads split across two independent DMA paths (each path executes
        # its DMAs serially FIFO): first half on the ACT HWDGE ring —
        # issued at trace start, before any relu occupies ACT's queue —
        # second half on the SWDGE (gpsimd) ring. Stores get the SP ring
        # to themselves.
        xtiles = []
        for bt in range(NBATCH):
            xt_t = xp.tile([P, 2 * ABATCH * N_B], BF16, tag="xt")
            dst = xt_t[:].rearrange("p (c a b) -> p c a b", c=2, a=ABATCH, b=N_B)
            src = xt_d[bt]  # [p, c, a, b]
            eng = nc.gpsimd
            if bt == 0:
                # split the first batch so compute starts ~2 us earlier
                eng.dma_start(dst[:, :, 0:2, :], src[:, :, 0:2, :])
                eng.dma_start(dst[:, :, 2:, :], src[:, :, 2:, :])
            else:
                eng.dma_start(dst, src)
            xtiles.append(xt_t)

        # --- main loop: block pairs, software-pipelined one pair deep -----
        # Per iteration (steady state), in trace order:
        #   1. PE: mm2 for the previous pair (h ready from last iteration)
        #   2. PE: mm1 accumulation for the current pair (weights loaded
        #      once per pair: wec0 x2 blocks, wec1 x2, wb x2)
        #   3. ACT/DVE: relu2(+be2) for prev pair -> ostage (+ store DMA)
        #   4. ACT/DVE: relu1(+aterm) for the current pair
        # so TensorE streams matmuls back-to-back (stays at K=8/8) while
        # the elementwise engines chew on the previous pair. relu work
        # alternates between ScalarE and VectorE per block.
        ostages = {}

        def act_relu(dst, src, bias_col):
            nc.scalar.activation(dst, src, AF.Relu, bias=bias_col)

        def dve_relu(dst, src, bias_col):
            nc.vector.tensor_scalar(dst, src, bias_col, 0.0,
                                    mybir.AluOpType.add, mybir.AluOpType.max)

        def emit_mm2(a, h):
            ps2 = ps.tile([P, N_B], F32, tag="ps", name=f"ps2_{a}")
            nc.tensor.matmul(ps2[:], w_t["we2"], h[:], start=True, stop=True)
            return ps2

        def emit_relu2(a, ps2):
            g, j = divmod(a, GROUP)
            if g not in ostages:
                ostages[g] = op.tile([P, GROUP * N_B], BF16, tag="ost", name=f"ost{g}")
            osl = ostages[g][:, j * N_B:(j + 1) * N_B]
            (act_relu if a % 2 == 0 else dve_relu)(osl, ps2[:], be2_t)
            if j == GROUP - 1:
                # stores ride the otherwise-idle SP ring (loads own the
                # SWDGE ring, relus own ACT's queue)
                nc.sync.dma_start(out_el_d[g].rearrange("p j b -> p (j b)"),
                                  ostages[g][:])

        def emit_relu1(a, ps1):
            h = hp.tile([P, N_B], BF16, tag="h", name=f"h{a}")
            (act_relu if a % 2 == 1 else dve_relu)(h[:], ps1[:], aterm[:, a:a + 1])
            return h

        from collections import deque
        pending = deque()  # pairs awaiting mm2+relu2; each = [(a, h), (a, h)]

        def retire_pair():
            old = pending.popleft()
            ps2s = [emit_mm2(a, h) for a, h in old]
            return old, ps2s

        for pr in range(ng * GROUP // 2):
            a0 = 2 * pr
            # 1. a SKEW-old pair's mm2 first — PE work whose h has long
            #    been ready, so PE never waits on the elementwise engines.
            old = retire_pair() if len(pending) == SKEW else None
            # 2. current pair's mm1 accumulation, weights shared per pair.
            ps1s = [ps.tile([P, N_B], F32, tag="ps", name=f"ps1_{a0 + i}")
                    for i in range(2)]
            for wi, wname in enumerate(["wec0", "wec1"]):
                for bi, a in enumerate((a0, a0 + 1)):
                    bt, aa = divmod(a, ABATCH)
                    rhs = xtiles[bt][:, (wi * ABATCH + aa) * N_B:
                                     (wi * ABATCH + aa + 1) * N_B]
                    nc.tensor.matmul(ps1s[bi][:], w_t[wname], rhs,
                                     start=(wi == 0), stop=False)
            for bi, a in enumerate((a0, a0 + 1)):
                nc.tensor.matmul(ps1s[bi][:], w_t["wb"], nbt_t,
                                 start=False, stop=True)
            # 3. the retired pair's relu2 (+ store when a group completes).
            if old:
                for (a, _), ps2 in zip(old[0], old[1]):
                    emit_relu2(a, ps2)
            # 4. current pair's relu1.
            pending.append([(a, emit_relu1(a, ps1s[bi]))
                            for bi, a in enumerate((a0, a0 + 1))])
        # epilogue: drain the pipeline
        while pending:
            old = retire_pair()
            for (a, _), ps2 in zip(old[0], old[1]):
                emit_relu2(a, ps2)

    nc.compile()
    return nc


def _get_program() -> bass.Bass:
    global _PROGRAM
    if _PROGRAM is None:
        _PROGRAM = build_program()
    return _PROGRAM


def make_in_maps(inputs: dict) -> list[dict]:
    bf = ml_dtypes.bfloat16
    E = np.asarray(inputs["edge_embeds"], np.float32)
    na_e = np.asarray(inputs["nodes_a_embeds"], np.float32)
    nb_e = np.asarray(inputs["nodes_b_embeds"], np.float32)
    We1 = np.asarray(inputs["We1"], np.float32)
    We2 = np.asarray(inputs["We2"], np.float32)
    be1 = np.asarray(inputs["be1"], np.float32)
    be2 = np.asarray(inputs["be2"], np.float32)

    # [core, bt, aa, b, c, p] -> [core, bt, p, c, aa, b]  (partition-major)
    Ebf = np.ascontiguousarray(
        E.astype(bf).reshape(NCORES, NBATCH, ABATCH, N_B, 2, P).transpose(0, 1, 5, 4, 2, 3)
    )

    # Weights are stored [in, out]; matmul lhsT wants [K=in on partitions,
    # M=out] — exactly W as stored, so wpack columns are the raw slices.
    wpack = np.concatenate(
        [We1[256:384], We1[384:512], We1[128:256], We2, nb_e.T], axis=1
    ).astype(bf)

    # aterm[:, a] = (nodes_a @ Wa + be1).T column a — fp32 per-block bias.
    aterm_all = (na_e @ We1[0:128] + be1).astype(np.float32)  # [N_A, D]

    def fpack_for(core):
        at = aterm_all[core * SH:(core + 1) * SH].T  # [P, SH]
        return np.ascontiguousarray(
            np.concatenate([at, be2.reshape(P, 1)], axis=1), np.float32
        )

    return [{"xt": Ebf[k], "wpack": wpack, "fpack": fpack_for(k)}
            for k in range(NCORES)]


def assemble_outputs(inputs: dict, results: list[dict]):
    na_e = np.asarray(inputs["nodes_a_embeds"], np.float32)
    nb_e = np.asarray(inputs["nodes_b_embeds"], np.float32)
    Wn1 = np.asarray(inputs["Wn1"], np.float32)
    Wn2 = np.asarray(inputs["Wn2"], np.float32)
    bn1 = np.asarray(inputs["bn1"], np.float32)
    bn2 = np.asarray(inputs["bn2"], np.float32)

    edge_latent = np.empty((N_A, N_B, D), np.float32)
    for k in range(NCORES):
        el = np.asarray(results[k]["out_el"], np.float32)  # [NG, P, GROUP, N_B]
        edge_latent[k * SH:(k + 1) * SH] = (
            el.transpose(0, 2, 3, 1).reshape(SH, N_B, D)
        )

    # Node updates in exact fp32 on the assembled tensor (the cross-shard
    # sum over a IS the all-reduce from the sharding hint).
    sum_a = edge_latent.sum(axis=1)  # [N_A, D]
    sum_b = edge_latent.sum(axis=0)  # [N_B, D]

    def node_mlp(x):
        h = np.maximum(x @ Wn1 + bn1, 0.0)
        return np.maximum(h @ Wn2 + bn2, 0.0).astype(np.float32)

    nodes_a = node_mlp(np.concatenate([na_e, sum_a], axis=1))
    nodes_b = node_mlp(np.concatenate([nb_e, sum_b], axis=1))
    return edge_latent, nodes_a, nodes_b


def kernel(**inputs):
    global LAST_RESULTS
    from concourse.bass_utils import run_bass_kernel_spmd

    nc = _get_program()
    in_maps = make_in_maps(inputs)
    res = run_bass_kernel_spmd(nc, in_maps, core_ids=list(range(NCORES)))
    LAST_RESULTS = res
    return assemble_outputs(inputs, res.results)
